# revision 14
# baseline (speedup 1.0000x reference)
"""BEV deformable cross-attention kernel for 8 Trainium2 NeuronCores.

Strategy (per core): data-parallel over (B x K-half): core c handles batch
b = c//2 and modes k in {3*(c%2) .. +3}, i.e. 36 queries, 288 sample points.

Key algebraic move: grid_sample(conv1x1(bev)) == conv1x1(grid_sample(bev)),
so instead of materializing the two full (256,200,200) conv maps we gather
only the 4 bilinear corners of the 288 sample points from a host-transposed
HWC copy of bev_feat (channels contiguous per pixel -> 2KB indirect reads),
interpolate in 256-d, then apply the 1x1 convs to 288 vectors.

Everything else (offset MLP, DAB-style sine embeddings with on-device range
reduction, positional MLPs, 8-key-per-query attention via selection-matrix
matmuls, output projection + residual) runs on-device in fp32, feature-major
(features on partitions, queries/points on the free axis).
"""
import numpy as np

import concourse.bass as bass
import concourse.mybir as mybir
import concourse.tile as tile_mod
from concourse.bass import AP, IndirectOffsetOnAxis

F32 = mybir.dt.float32
I32 = mybir.dt.int32
AF = mybir.ActivationFunctionType
OP = mybir.AluOpType

# problem constants (hardcoded per contract)
K, B, T, DIM = 6, 4, 12, 256
H, W = 200, 200
HALF = 256
G = 8                      # offset groups == sample points per query
NH = 8                     # heads
HD = 32                    # head dim of value part
NQ = 3 * T                 # queries per core = 36
NPT = NQ * G               # points per core = 288
OFFSET_SCALE = 4.0
PIX_SCALE = float(W / 102.4)          # 1.953125
PIX_BIAS = float(W / 2.0 - 0.5)       # 99.5
SCALE = 64 ** -0.5                    # 0.125
TWO_PI = float(2 * np.pi)
RC = float(3 * 2 ** 22)               # 1.5*2^23 rint magic constant
CHUNKS = [(0, 128), (128, 128), (256, 32)]   # point chunks (start, size)

# ---------------------------------------------------------------- blob layout


class Alloc:
    def __init__(self):
        self.pos = 0
        self.slices = {}

    def add(self, name, width):
        self.slices[name] = (self.pos, width)
        self.pos += width

    def __getitem__(self, name):
        return self.slices[name]


def wblob_layout():
    a = Alloc()
    for nm, wd in [
        ("wconq", 512), ("bconq", 2), ("bdh", 512), ("bo1rep", 1),
        ("wo2top", 2), ("wo2bot", 2), ("bo2", 1), ("sc2pm", 2),
        ("freq", 128), ("freqx2", 128), ("freqy2", 128),
        ("shift", 128), ("ones", 288), ("ident", 128),
        ("wq1", 512), ("bq1", 2), ("wq2", 512), ("bq2", 2),
        ("wk1", 512), ("bk1", 2), ("wk2", 512), ("bk2", 2),
        ("wcat", 1024), ("wout", 512), ("bout", 2),
        ("s0", 8), ("s1", 8), ("e0", 128), ("e1", 128),
    ]:
        a.add(nm, wd)
    return a


def xblob_layout():
    a = Alloc()
    for nm, wd in [("deT", 72), ("qsT", 72), ("rpx", 36), ("rpy", 36),
                   ("rpexp", 288)]:
        a.add(nm, wd)
    return a


def pack_wblob(weights):
    """weights: dict of numpy arrays (original reference layouts)."""
    lay = wblob_layout()
    wb = np.zeros((128, lay.pos), np.float32)

    def put(name, arr, rows=128, coloff=0):
        s, _ = lay[name]
        wb[:rows, s + coloff: s + coloff + arr.shape[1]] = arr

    def put_mm(name, w256):  # (256, Mout) -> blocks (kc, mc) of (128, 128)
        s, _ = lay[name]
        mcs = w256.shape[1] // 128
        for kc in range(2):
            for mc in range(mcs):
                blk = w256[kc * 128:(kc + 1) * 128, mc * 128:(mc + 1) * 128]
                off = (kc * mcs + mc) * 128
                wb[:, s + off: s + off + 128] = blk

    put_mm("wconq", weights["W_con_q"])
    put("bconq", weights["b_con_q"].reshape(2, 128).T)
    # block-diag Wo1 consts: j = cc*2+h2 covers groups (2j, 2j+1)
    s, _ = lay["bdh"]
    wo1 = weights["Wo1"]  # (32, 64)
    for j in range(4):
        blk = np.zeros((128, 128), np.float32)
        blk[0:32, 0:64] = wo1 if j % 2 == 0 else 0
        if j % 2 == 0:
            blk[0:32, 0:64] = wo1
            blk[32:64, 64:128] = wo1
        else:
            blk[64:96, 0:64] = wo1
            blk[96:128, 64:128] = wo1
        wb[:, s + j * 128: s + (j + 1) * 128] = blk
    put("bo1rep", np.tile(weights["bo1"], 2)[:, None])
    wo2 = weights["Wo2"]  # (64, 2)
    top = np.zeros((128, 2), np.float32); top[0:64] = wo2
    bot = np.zeros((128, 2), np.float32); bot[64:128] = wo2
    put("wo2top", top); put("wo2bot", bot)
    put("bo2", weights["bo2"][:, None], rows=2)
    put("sc2pm", np.tile(np.array([[PIX_SCALE, -PIX_SCALE]], np.float32),
                         (128, 1)))
    i64 = np.arange(128) // 2
    freq = (TWO_PI / (10000.0 ** (i64 / 64.0))).astype(np.float32)
    put("freq", freq[None, :], rows=1)
    fx2 = np.zeros((2, 128), np.float32); fx2[0] = freq
    fy2 = np.zeros((2, 128), np.float32); fy2[1] = freq
    put("freqx2", fx2, rows=2)
    put("freqy2", fy2, rows=2)
    shift = np.where(np.arange(128) % 2 == 1, np.pi / 2, 0.0).astype(np.float32)
    put("shift", shift[None, :], rows=1)
    put("ones", np.ones((1, 288), np.float32), rows=1)
    put("ident", np.eye(128, dtype=np.float32))
    put_mm("wq1", weights["Wq1"]); put("bq1", weights["bq1"].reshape(2, 128).T)
    put_mm("wq2", weights["Wq2"]); put("bq2", weights["bq2"].reshape(2, 128).T)
    put_mm("wk1", weights["Wk1"]); put("bk1", weights["bk1"].reshape(2, 128).T)
    put_mm("wk2", weights["Wk2"]); put("bk2", weights["bk2"].reshape(2, 128).T)
    wcat = np.concatenate([weights["W_con_k"], weights["W_v"]], axis=1)  # (256,512)
    put_mm("wcat", wcat)
    put_mm("wout", weights["W_out"])
    put("bout", weights["b_out"].reshape(2, 128).T)
    d = np.arange(128)
    s0 = np.zeros((128, 8), np.float32)
    s0[d, d // 32] = SCALE
    s1 = np.zeros((128, 8), np.float32)
    s1[d, 4 + d // 32] = SCALE
    put("s0", s0); put("s1", s1)
    e0 = np.zeros((8, 128), np.float32)
    e0[d // 32, d] = 1.0
    e1 = np.zeros((8, 128), np.float32)
    e1[4 + d // 32, d] = 1.0
    put("e0", e0, rows=8); put("e1", e1, rows=8)
    return wb


def pack_xblob(dec_embed, query_scale, ref_points, b, k0):
    """Per-core input blob: 36 queries = modes k0..k0+2, all T."""
    lay = xblob_layout()
    xb = np.zeros((128, lay.pos), np.float32)
    de = dec_embed[k0:k0 + 3, b].reshape(NQ, DIM)       # (36, 256)
    qs = query_scale[k0:k0 + 3, b].reshape(NQ, DIM)
    rp = ref_points[k0:k0 + 3, b].reshape(NQ, 2)

    s, _ = lay["deT"]
    xb[:, s: s + 36] = de.T[:128]
    xb[:, s + 36: s + 72] = de.T[128:]
    s, _ = lay["qsT"]
    xb[:, s: s + 36] = qs.T[:128]
    xb[:, s + 36: s + 72] = qs.T[128:]
    s, _ = lay["rpx"]
    xb[0, s: s + 36] = rp[:, 0]
    s, _ = lay["rpy"]
    xb[0, s: s + 36] = rp[:, 1]
    s, _ = lay["rpexp"]
    xb[0:2, s: s + 288] = np.tile(rp.T, (1, 8))         # g-major: col = g*36+q
    return xb


# --------------------------------------------------------------- tile patches

def _split_drain_and_barrier(self, tick_clock, wait_clock):
    nc = self.nc
    drain_inst = nc.sync.drain()
    wait_clock.add_sem_waits(
        drain_inst.ins, tile_mod.ScopedClock({None: tick_clock.global_clock})
    )
    si = drain_inst.ins.sync_info
    waits = list(si.on_wait)
    if len(waits) > 1:
        si.on_wait = waits[:1]
        for i in range(1, len(waits)):
            extra = nc.sync.drain()
            extra.ins.sync_info = type(si)(on_wait=waits[i: i + 1], on_update=[])
    nc.all_engine_barrier()
    assert self.sems is not None
    popped = nc._tile_sem_poison_stack.pop()
    assert popped is self._sem_poison
    nc.clear_and_free_semaphores(list(self.sems.allocated().values()))
    nc.all_engine_barrier()


def split_multiwaits(nc):
    """walrus codegen supports a single sync-wait per instruction; split."""
    f = nc.m.functions[0]
    for blk in f.blocks:
        todo = [i for i in blk.instructions
                if i.sync_info is not None and len(i.sync_info.on_wait) > 1]
        for inst in todo:
            si = inst.sync_info
            waits = list(si.on_wait)
            nops = []
            for w in waits[:-1]:
                bi = nc.engines[inst.engine].nop(nofuse=True)
                ni = bi.ins
                for b2 in f.blocks:
                    if b2.instructions and b2.instructions[-1] is ni:
                        b2.instructions.pop()
                        break
                ni.sync_info = type(si)(on_wait=[w], on_update=[])
                nops.append(ni)
            si.on_wait = [waits[-1]]
            pos = blk.instructions.index(inst)
            blk.instructions[pos:pos] = nops


_PATCHED = False


def patch_tile():
    global _PATCHED
    if not _PATCHED:
        tile_mod.TileContext._drain_and_barrier = _split_drain_and_barrier
        _PATCHED = True


# ---------------------------------------------------------------- the kernel

def view3(ap, dims):
    """Build a 3D AP view on top of a 2D tile AP: dims = [[step,count],...]
    applied after the partition dim (ap.ap[0] kept)."""
    return AP(ap.tensor, ap.offset, [ap.ap[0]] + dims)


def build_nc(sim_mode=False, debug=False):
    patch_tile()
    nc = bass.Bass("TRN2")
    wlay = wblob_layout()
    xlay = xblob_layout()

    bev = nc.dram_tensor("bev", [H * W, 256], F32, kind="ExternalInput")
    wbl = nc.dram_tensor("wbl", [128, wlay.pos], F32, kind="ExternalInput")
    xbl = nc.dram_tensor("xbl", [128, xlay.pos], F32, kind="ExternalInput")
    out = nc.dram_tensor("out", [256, NQ], F32, kind="ExternalOutput")

    dbg = {}
    if debug:
        for nm, shp, dt in [
            ("d_vg", [2, 288], F32), ("d_pix", [128, 2], F32),
            ("d_idx", [128, 1], I32), ("d_sam0", [128, 256], F32),
            ("d_sim", [8, 288], F32), ("d_at", [8, 288], F32),
            ("d_kse0", [128, 288], F32), ("d_posk0", [128, 288], F32),
            ("d_conv0", [128, 288], F32), ("d_qse0", [128, 36], F32),
            ("d_cq0", [128, 36], F32), ("d_h", [128, 144], F32),
            ("d_av0", [128, 36], F32), ("d_w40", [128, 4], F32),
        ]:
            dbg[nm] = nc.dram_tensor(nm, shp, dt, kind="ExternalOutput")

    ERF = AF.Sigmoid if sim_mode else AF.Erf

    with tile_mod.TileContext(nc) as tc:
        with (
            tc.tile_pool(name="sbuf", bufs=1) as pool,
            tc.tile_pool(name="psum", bufs=1, space="PSUM") as psum,
        ):
            wb = pool.tile([128, wlay.pos], F32)
            nc.sync.dma_start(out=wb[:], in_=wbl[:])
            xb = pool.tile([128, xlay.pos], F32)
            nc.sync.dma_start(out=xb[:], in_=xbl[:])

            def wsl(name, rows=128, off=0, width=None):
                s, wd = wlay[name]
                if width is None:
                    width = wd - off
                return wb[0:rows, s + off: s + off + width]

            def xsl(name, rows=128, off=0, width=None):
                s, wd = xlay[name]
                if width is None:
                    width = wd - off
                return xb[0:rows, s + off: s + off + width]

            deT = [xsl("deT", off=mc * 36, width=36) for mc in range(2)]
            qsT = [xsl("qsT", off=mc * 36, width=36) for mc in range(2)]

            # ---- 1. con_q = de @ W_con_q + b  (feature-major, 2 chunks)
            cqS = []
            for mc in range(2):
                p = psum.tile([128, 288], F32, space="PSUM", tag="psA", bufs=3, name="cqP")
                for kc in range(2):
                    nc.tensor.matmul(
                        out=p[:, :36], lhsT=wsl("wconq", off=(kc * 2 + mc) * 128, width=128),
                        rhs=deT[kc], start=(kc == 0), stop=(kc == 1))
                t = pool.tile([128, 36], F32, tag=f"cqS{mc}")
                nc.scalar.activation(out=t[:], in_=p[:, :36], func=AF.Identity,
                                     bias=wsl("bconq", off=mc, width=1))
                cqS.append(t)
            if debug:
                nc.sync.dma_start(out=dbg["d_cq0"][:], in_=cqS[0][:])

            # ---- 2. h = gelu(grouped con_q @ Wo1 + bo1): 4 block-diag mms
            hP = psum.tile([128, 288], F32, space="PSUM", tag="psA", bufs=3, name="hP")
            for j in range(4):
                cc = j // 2
                nc.tensor.matmul(
                    out=hP[:, j * 36:(j + 1) * 36],
                    lhsT=wsl("bdh", off=j * 128, width=128),
                    rhs=cqS[cc][:], start=True, stop=True)
            hx = pool.tile([128, 144], F32)
            nc.scalar.activation(out=hx[:], in_=hP[:, :144], func=AF.Identity,
                                 bias=wsl("bo1rep"))
            he = pool.tile([128, 144], F32)
            nc.scalar.activation(out=he[:], in_=hx[:], func=ERF,
                                 scale=float(1 / np.sqrt(2)), bias=0.0)
            nc.vector.tensor_scalar(out=he[:], in0=he[:], scalar1=0.5,
                                    scalar2=0.5, op0=OP.mult, op1=OP.add)
            hS = pool.tile([128, 144], F32)
            nc.vector.tensor_tensor(out=hS[:], in0=hx[:], in1=he[:], op=OP.mult)
            if debug:
                nc.sync.dma_start(out=dbg["d_h"][:], in_=hS[:])

            # ---- 3. offsets + vgrid (meters), g-major (2, 288)
            offP = psum.tile([2, 288], F32, space="PSUM", tag="psA", bufs=3, name="offP")
            for g in range(8):
                j = g // 2
                lhs = wsl("wo2top", width=2) if g % 2 == 0 else wsl("wo2bot", width=2)
                nc.tensor.matmul(out=offP[:, g * 36:(g + 1) * 36], lhsT=lhs,
                                 rhs=hS[:, j * 36:(j + 1) * 36],
                                 start=True, stop=True)
            tof = pool.tile([2, 288], F32)
            nc.scalar.activation(out=tof[:], in_=offP[:], func=AF.Tanh,
                                 bias=wsl("bo2", rows=2, width=1))
            vgT = pool.tile([2, 288], F32)
            nc.vector.tensor_scalar(out=vgT[:], in0=tof[:], scalar1=OFFSET_SCALE,
                                    scalar2=None, op0=OP.mult)
            nc.vector.tensor_tensor(out=vgT[:], in0=vgT[:],
                                    in1=xsl("rpexp", rows=2), op=OP.add)
            if debug:
                nc.sync.dma_start(out=dbg["d_vg"][:], in_=vgT[:])

            # ---- 4+5. transpose vgrid to point-major, then per-point geometry
            # (all per-point scalars live in columns of the same partition)
            idxI, w4, pixdbg = [], [], None
            for c, (c0, cn) in enumerate(CHUNKS):
                tp = psum.tile([128, 2], F32, space="PSUM", tag="psA", bufs=3, name="tpP")
                nc.tensor.transpose(out=tp[:cn, :], in_=vgT[:, c0:c0 + cn],
                                    identity=wsl("ident", rows=2, width=2))
                # pix = vg * [s, -s] + 99.5   (cols [gx, gy])
                pix = pool.tile([128, 2], F32, tag=f"pix{c}")
                nc.vector.tensor_tensor(out=pix[:cn, :], in0=tp[:cn, :],
                                        in1=wsl("sc2pm", rows=cn, width=2),
                                        op=OP.mult)
                nc.vector.tensor_scalar(out=pix[:cn, :], in0=pix[:cn, :],
                                        scalar1=PIX_BIAS, scalar2=None,
                                        op0=OP.add)
                # f0 = rint(pix - 0.5) = floor(pix) via the 1.5*2^23 trick
                f0 = pool.tile([128, 2], F32, tag=f"f0{c}")
                nc.vector.tensor_scalar(out=f0[:cn, :], in0=pix[:cn, :],
                                        scalar1=-0.5, scalar2=float(RC),
                                        op0=OP.add, op1=OP.add)
                nc.vector.tensor_scalar(out=f0[:cn, :], in0=f0[:cn, :],
                                        scalar1=float(-RC), scalar2=None,
                                        op0=OP.add)
                fr = pool.tile([128, 2], F32, tag=f"fr{c}")
                nc.vector.tensor_tensor(out=fr[:cn, :], in0=pix[:cn, :],
                                        in1=f0[:cn, :], op=OP.subtract)
                # idx = y0*200 + x0 (float-exact, then cast)
                idf = pool.tile([128, 1], F32, tag=f"idf{c}")
                nc.vector.tensor_scalar(out=idf[:cn, :], in0=f0[:cn, 1:2],
                                        scalar1=float(W), scalar2=None,
                                        op0=OP.mult)
                nc.vector.tensor_tensor(out=idf[:cn, :], in0=idf[:cn, :],
                                        in1=f0[:cn, 0:1], op=OP.add)
                ii = pool.tile([128, 1], I32, tag=f"idxI{c}")
                nc.vector.tensor_copy(out=ii[:cn, :], in_=idf[:cn, :])
                idxI.append(ii)
                # bilinear weights (Pc, 4) = [w00, w10, w01, w11]
                wxp = pool.tile([128, 2], F32, tag=f"wxp{c}")
                nc.vector.tensor_scalar(out=wxp[:cn, 0:1], in0=fr[:cn, 0:1],
                                        scalar1=-1.0, scalar2=1.0,
                                        op0=OP.mult, op1=OP.add)
                nc.scalar.copy(out=wxp[:cn, 1:2], in_=fr[:cn, 0:1])
                wyp = pool.tile([128, 2], F32, tag=f"wyp{c}")
                nc.vector.tensor_scalar(out=wyp[:cn, 0:1], in0=fr[:cn, 1:2],
                                        scalar1=-1.0, scalar2=1.0,
                                        op0=OP.mult, op1=OP.add)
                nc.scalar.copy(out=wyp[:cn, 1:2], in_=fr[:cn, 1:2])
                w4c = pool.tile([128, 4], F32, tag=f"w4{c}")
                wxa = wxp[:cn, :]
                wya = wyp[:cn, :]
                nc.vector.tensor_tensor(
                    out=view3(w4c[:cn, :], [[2, 2], [1, 2]]),
                    in0=AP(wxa.tensor, wxa.offset, [wxa.ap[0], [0, 2], [1, 2]]),
                    in1=AP(wya.tensor, wya.offset, [wya.ap[0], [1, 2], [0, 2]]),
                    op=OP.mult)
                w4.append(w4c)
                if debug and c == 0:
                    pixdbg = pix
            if debug:
                nc.sync.dma_start(out=dbg["d_pix"][:], in_=pixdbg[:])
                nc.sync.dma_start(out=dbg["d_idx"][:], in_=idxI[0][:])
                nc.sync.dma_start(out=dbg["d_w40"][:], in_=w4[0][:])

            # ---- 6. gathers: 2KB rows y0 / y0+1 per point
            gA, gB = [], []
            for c, (c0, cn) in enumerate(CHUNKS):
                ga = pool.tile([128, 512], F32, tag=f"gA{c}")
                nc.gpsimd.indirect_dma_start(
                    out=ga[:cn, :], out_offset=None, in_=bev[:],
                    in_offset=IndirectOffsetOnAxis(ap=idxI[c][:cn, :], axis=0))
                gb = pool.tile([128, 512], F32, tag=f"gB{c}")
                nc.gpsimd.indirect_dma_start(
                    out=gb[:cn, :], out_offset=None, in_=bev[:],
                    in_offset=IndirectOffsetOnAxis(ap=idxI[c][:cn, :], axis=0),
                    element_offset=W * 256)
                gA.append(ga); gB.append(gb)

            # ---- 7. bilinear combine -> sampled (point-major)
            sam = []
            for c, (c0, cn) in enumerate(CHUNKS):
                t1 = pool.tile([128, 256], F32, tag=f"bt1{c}")
                t2 = pool.tile([128, 256], F32, tag=f"bt2{c}")
                sm = pool.tile([128, 256], F32, tag=f"sam{c}")
                nc.scalar.activation(out=t1[:cn, :], in_=gA[c][:cn, 0:256],
                                     func=AF.Copy, scale=w4[c][:cn, 0:1])
                nc.vector.tensor_scalar(out=t2[:cn, :], in0=gA[c][:cn, 256:512],
                                        scalar1=w4[c][:cn, 1:2], scalar2=None,
                                        op0=OP.mult)
                nc.vector.tensor_tensor(out=t1[:cn, :], in0=t1[:cn, :],
                                        in1=t2[:cn, :], op=OP.add)
                nc.scalar.activation(out=t2[:cn, :], in_=gB[c][:cn, 0:256],
                                     func=AF.Copy, scale=w4[c][:cn, 2:3])
                nc.vector.tensor_tensor(out=t1[:cn, :], in0=t1[:cn, :],
                                        in1=t2[:cn, :], op=OP.add)
                nc.vector.tensor_scalar(out=t2[:cn, :], in0=gB[c][:cn, 256:512],
                                        scalar1=w4[c][:cn, 3:4], scalar2=None,
                                        op0=OP.mult)
                nc.vector.tensor_tensor(out=sm[:cn, :], in0=t1[:cn, :],
                                        in1=t2[:cn, :], op=OP.add)
                sam.append(sm)
            if debug:
                nc.sync.dma_start(out=dbg["d_sam0"][:], in_=sam[0][:])

            # ---- 8. transpose sampled to feature-major (256, 288) = 2 tiles
            samT = [pool.tile([128, 288], F32, tag=f"samT{fc}", name=f"samT{fc}")
                    for fc in range(2)]
            for c, (c0, cn) in enumerate(CHUNKS):
                for fc in range(2):
                    tp = psum.tile([128, 128], F32, space="PSUM", tag="psA", bufs=3, name="samtp")
                    nc.tensor.transpose(out=tp[:, :cn],
                                        in_=sam[c][:cn, fc * 128:(fc + 1) * 128],
                                        identity=wsl("ident", rows=cn, width=cn))
                    nc.scalar.copy(out=samT[fc][:, c0:c0 + cn], in_=tp[:, :cn])

            # ---- 9. conv: con_k / v = sampled @ [W_con_k | W_v]
            convP = []
            for mc in range(4):
                p = psum.tile([128, 288], F32, space="PSUM", tag="convP", bufs=4, name="convP")
                for kc in range(2):
                    nc.tensor.matmul(
                        out=p[:], lhsT=wsl("wcat", off=(kc * 4 + mc) * 128, width=128),
                        rhs=samT[kc][:], start=(kc == 0), stop=(kc == 1))
                convP.append(p)
            vS = []
            for fc in range(2):
                t = pool.tile([128, 288], F32, tag=f"vS{fc}")
                nc.scalar.copy(out=t[:], in_=convP[2 + fc][:])
                vS.append(t)
            if debug:
                t = pool.tile([128, 288], F32)
                nc.scalar.copy(out=t[:], in_=convP[0][:])
                nc.sync.dma_start(out=dbg["d_conv0"][:], in_=t[:])

            # ---- helpers: sine embedding (feature-major halves)
            def sine_embed(lhs_name, lhs_rows, rhs_ap, n, tag):
                """phase = freq (x) meters + shift; one 128-row half."""
                ph = psum.tile([128, 288], F32, space="PSUM", tag="psA", bufs=3, name="phP")
                nc.tensor.matmul(out=ph[:, :n], lhsT=wsl(lhs_name, rows=lhs_rows),
                                 rhs=rhs_ap, start=True, stop=False)
                nc.tensor.matmul(out=ph[:, :n], lhsT=wsl("shift", rows=1),
                                 rhs=wsl("ones", rows=1, width=n),
                                 start=False, stop=True)
                m1t = pool.tile([128, n], F32, tag=f"sm1{tag}")
                nc.vector.tensor_scalar(out=m1t[:], in0=ph[:, :n],
                                        scalar1=float(1.0 / TWO_PI), scalar2=RC,
                                        op0=OP.mult, op1=OP.add)
                k2t = pool.tile([128, n], F32, tag=f"sk2{tag}")
                nc.vector.tensor_scalar(out=k2t[:], in0=m1t[:], scalar1=-RC,
                                        scalar2=-TWO_PI, op0=OP.add, op1=OP.mult)
                yt = pool.tile([128, n], F32, tag=f"sy{tag}")
                nc.vector.tensor_tensor(out=yt[:], in0=ph[:, :n], in1=k2t[:], op=OP.add)
                nc.vector.tensor_scalar(out=yt[:], in0=yt[:],
                                        scalar1=float(np.pi),
                                        scalar2=float(-np.pi),
                                        op0=OP.min, op1=OP.max)
                st = pool.tile([128, n], F32, tag=f"se{tag}")
                nc.scalar.activation(out=st[:], in_=yt[:], func=AF.Sin)
                return st

            def mlp2(inp2, n, wn1, bn1, wn2, bn2, tag):
                """two-layer MLP relu(x@W1+b1)@W2+b2, feature-major chunks."""
                mid = []
                for mc in range(2):
                    p = psum.tile([128, 288], F32, space="PSUM", tag="psA", bufs=3, name="m1P")
                    for kc in range(2):
                        nc.tensor.matmul(
                            out=p[:, :n], lhsT=wsl(wn1, off=(kc * 2 + mc) * 128, width=128),
                            rhs=inp2[kc][:], start=(kc == 0), stop=(kc == 1))
                    t = pool.tile([128, n], F32, tag=f"m1S{tag}{mc}")
                    nc.scalar.activation(out=t[:], in_=p[:, :n], func=AF.Relu,
                                         bias=wsl(bn1, off=mc, width=1))
                    mid.append(t)
                outs = []
                for mc in range(2):
                    p = psum.tile([128, 288], F32, space="PSUM", tag="psA", bufs=3, name="m2P")
                    for kc in range(2):
                        nc.tensor.matmul(
                            out=p[:, :n], lhsT=wsl(wn2, off=(kc * 2 + mc) * 128, width=128),
                            rhs=mid[kc][:], start=(kc == 0), stop=(kc == 1))
                    t = pool.tile([128, n], F32, tag=f"m2S{tag}{mc}")
                    nc.scalar.activation(out=t[:], in_=p[:, :n], func=AF.Identity,
                                         bias=wsl(bn2, off=mc, width=1))
                    outs.append(t)
                return outs

            # ---- 10. qse + pos_q (36 cols)  [emb(y) first, then emb(x)]
            qse = [sine_embed("freq", 1, xsl("rpy", rows=1), 36, "qy"),
                   sine_embed("freq", 1, xsl("rpx", rows=1), 36, "qx")]
            if debug:
                nc.sync.dma_start(out=dbg["d_qse0"][:], in_=qse[0][:])
            pqS = mlp2(qse, 36, "wq1", "bq1", "wq2", "bq2", "q")
            for mc in range(2):
                nc.vector.tensor_tensor(out=pqS[mc][:], in0=pqS[mc][:],
                                        in1=qsT[mc], op=OP.mult)

            # ---- 11. kse + pos_k (288 cols)
            kse = [sine_embed("freqy2", 2, vgT[:], 288, "ky"),
                   sine_embed("freqx2", 2, vgT[:], 288, "kx")]
            if debug:
                nc.sync.dma_start(out=dbg["d_kse0"][:], in_=kse[0][:])
            pkS = mlp2(kse, 288, "wk1", "bk1", "wk2", "bk2", "k")
            if debug:
                nc.sync.dma_start(out=dbg["d_posk0"][:], in_=pkS[0][:])

            # ---- 12. sim = scaled per-head dots via selection matmuls
            simP = psum.tile([8, 288], F32, space="PSUM", tag="simP", bufs=1, name="simP")
            pairs = [(convP[0], cqS[0], "s0"), (convP[1], cqS[1], "s1"),
                     (pkS[0], pqS[0], "s0"), (pkS[1], pqS[1], "s1")]
            for i, (kpart, qpart, sname) in enumerate(pairs):
                tmp = pool.tile([128, 288], F32, tag=f"tmp{i}")
                qap = qpart[:]
                nc.vector.tensor_tensor(
                    out=view3(tmp[:], [[36, 8], [1, 36]]),
                    in0=view3(kpart[:], [[36, 8], [1, 36]]),
                    in1=AP(qap.tensor, qap.offset, [qap.ap[0], [0, 8], [1, 36]]),
                    op=OP.mult)
                nc.tensor.matmul(out=simP[:], lhsT=wsl(sname, width=8),
                                 rhs=tmp[:], start=(i == 0), stop=(i == 3))
            if debug:
                t = pool.tile([8, 288], F32)
                nc.vector.tensor_copy(out=t[:], in_=simP[:])
                nc.sync.dma_start(out=dbg["d_sim"][:], in_=t[:])

            # ---- 13. softmax over g (stride-36 views; cols are g-major)
            mx = pool.tile([8, 36], F32)
            nc.vector.reduce_max(out=mx[:], in_=view3(simP[:], [[1, 36], [36, 8]]),
                                 axis=mybir.AxisListType.X)
            ex = pool.tile([8, 288], F32)
            mxa = mx[:]
            nc.vector.tensor_tensor(
                out=view3(ex[:], [[1, 36], [36, 8]]),
                in0=view3(simP[:], [[1, 36], [36, 8]]),
                in1=AP(mxa.tensor, mxa.offset, [mxa.ap[0], [1, 36], [0, 8]]),
                op=OP.subtract)
            nc.scalar.activation(out=ex[:], in_=ex[:], func=AF.Exp)
            sm = pool.tile([8, 36], F32)
            nc.vector.reduce_sum(out=sm[:], in_=view3(ex[:], [[1, 36], [36, 8]]),
                                 axis=mybir.AxisListType.X)
            rc = pool.tile([8, 36], F32)
            nc.vector.reciprocal(out=rc[:], in_=sm[:])
            at = pool.tile([8, 288], F32)
            rca = rc[:]
            nc.vector.tensor_tensor(
                out=view3(at[:], [[1, 36], [36, 8]]),
                in0=view3(ex[:], [[1, 36], [36, 8]]),
                in1=AP(rca.tensor, rca.offset, [rca.ap[0], [1, 36], [0, 8]]),
                op=OP.mult)
            if debug:
                nc.sync.dma_start(out=dbg["d_at"][:], in_=at[:])

            # ---- 14. attn-weighted values -> (256, 36) feature-major
            avT = []
            for fc in range(2):
                ae = psum.tile([128, 288], F32, space="PSUM", tag="psA", bufs=3, name="aeP")
                nc.tensor.matmul(out=ae[:], lhsT=wsl(f"e{fc}", rows=8, width=128),
                                 rhs=at[:], start=True, stop=True)
                pr = pool.tile([128, 288], F32, tag=f"pr{fc}")
                nc.vector.tensor_tensor(out=pr[:], in0=vS[fc][:], in1=ae[:],
                                        op=OP.mult)
                av = pool.tile([128, 36], F32, tag=f"avT{fc}")
                nc.vector.reduce_sum(out=av[:], in_=view3(pr[:], [[1, 36], [36, 8]]),
                                     axis=mybir.AxisListType.X)
                avT.append(av)
            if debug:
                nc.sync.dma_start(out=dbg["d_av0"][:], in_=avT[0][:])

            # ---- 15. out = attn_out @ W_out + b_out + identity
            for mc in range(2):
                p = psum.tile([128, 288], F32, space="PSUM", tag="psA", bufs=3, name="oP")
                for kc in range(2):
                    nc.tensor.matmul(
                        out=p[:, :36], lhsT=wsl("wout", off=(kc * 2 + mc) * 128, width=128),
                        rhs=avT[kc][:], start=(kc == 0), stop=(kc == 1))
                t = pool.tile([128, 36], F32, tag=f"oS{mc}")
                nc.scalar.activation(out=t[:], in_=p[:, :36], func=AF.Identity,
                                     bias=wsl("bout", off=mc, width=1))
                nc.vector.tensor_tensor(out=t[:], in0=t[:], in1=deT[mc], op=OP.add)
                nc.sync.dma_start(out=out[mc * 128:(mc + 1) * 128, :], in_=t[:])

    return nc


# ------------------------------------------------------------------- driver

def make_in_maps(dec_embed, bev_feat, query_scale, ref_points, weights):
    wb = pack_wblob(weights)
    in_maps = []
    for c in range(8):
        b, kh = c // 2, c % 2
        bev_hwc = np.ascontiguousarray(
            bev_feat[b].transpose(1, 2, 0).reshape(H * W, 256))
        xb = pack_xblob(dec_embed, query_scale, ref_points, b, 3 * kh)
        in_maps.append({"bev": bev_hwc, "wbl": wb, "xbl": xb})
    return in_maps


def assemble_output(results, dec_dtype=np.float32):
    out = np.zeros((K, B, T, DIM), np.float32)
    for c in range(8):
        b, kh = c // 2, c % 2
        oc = results[c]["out"]                     # (256, 36)
        out[3 * kh:3 * kh + 3, b] = oc.T.reshape(3, T, DIM)
    return out


_WNAMES = ["W_con_q", "b_con_q", "W_con_k", "W_v", "Wq1", "bq1", "Wq2", "bq2",
           "Wk1", "bk1", "Wk2", "bk2", "Wo1", "bo1", "Wo2", "bo2",
           "W_out", "b_out"]


def kernel(**inputs):
    from concourse.bass_utils import run_bass_kernel_spmd
    dec_embed = np.asarray(inputs["dec_embed"], np.float32)
    bev_feat = np.asarray(inputs["bev_feat"], np.float32)
    query_scale = np.asarray(inputs["query_scale"], np.float32)
    ref_points = np.asarray(inputs["ref_points"], np.float32)
    weights = {n: np.asarray(inputs[n], np.float32) for n in _WNAMES}

    nc = build_nc(sim_mode=False, debug=False)
    split_multiwaits(nc)
    in_maps = make_in_maps(dec_embed, bev_feat, query_scale, ref_points, weights)
    res = run_bass_kernel_spmd(nc, in_maps, list(range(8)))
    return assemble_output(res.results)


# revision 19
# speedup vs baseline: 1.1009x; 1.1009x over previous
"""BEV deformable cross-attention kernel for 8 Trainium2 NeuronCores.

Strategy (per core): data-parallel over (B x K-half): core c handles batch
b = c//2 and modes k in {3*(c%2) .. +3}, i.e. 36 queries, 288 sample points.

Key algebraic move: grid_sample(conv1x1(bev)) == conv1x1(grid_sample(bev)),
so instead of materializing the two full (256,200,200) conv maps we gather
only the 4 bilinear corners of the 288 sample points from a host-transposed
HWC copy of bev_feat (channels contiguous per pixel -> 2KB indirect reads),
interpolate in 256-d, then apply the 1x1 convs to 288 vectors.

Everything else (offset MLP, DAB-style sine embeddings with on-device range
reduction, positional MLPs, 8-key-per-query attention via selection-matrix
matmuls, output projection + residual) runs on-device in fp32, feature-major
(features on partitions, queries/points on the free axis).
"""
import numpy as np

import concourse.bass as bass
import concourse.mybir as mybir
import concourse.tile as tile_mod
from concourse.bass import AP, IndirectOffsetOnAxis

F32 = mybir.dt.float32
F32R = mybir.dt.float32r
I32 = mybir.dt.int32
AF = mybir.ActivationFunctionType
OP = mybir.AluOpType

# problem constants (hardcoded per contract)
K, B, T, DIM = 6, 4, 12, 256
H, W = 200, 200
HALF = 256
G = 8                      # offset groups == sample points per query
NH = 8                     # heads
HD = 32                    # head dim of value part
NQ = 3 * T                 # queries per core = 36
NPT = NQ * G               # points per core = 288
OFFSET_SCALE = 4.0
PIX_SCALE = float(W / 102.4)          # 1.953125
PIX_BIAS = float(W / 2.0 - 0.5)       # 99.5
SCALE = 64 ** -0.5                    # 0.125
TWO_PI = float(2 * np.pi)
RC = float(3 * 2 ** 22)               # 1.5*2^23 rint magic constant
CHUNKS = [(0, 128), (128, 128), (256, 32)]   # point chunks (start, size)

# ---------------------------------------------------------------- blob layout


class Alloc:
    def __init__(self):
        self.pos = 0
        self.slices = {}

    def add(self, name, width):
        self.slices[name] = (self.pos, width)
        self.pos += width

    def __getitem__(self, name):
        return self.slices[name]


WBLOBS = {
    # critical path first (fp32)
    "A": [("wconq", 512), ("bconq", 2), ("bdh", 512), ("bo1rep", 1),
          ("wo2top", 2), ("wo2bot", 2), ("bo2", 1), ("sc2pm", 2),
          ("freq", 128), ("freqx2", 128), ("freqy2", 128),
          ("shift", 128), ("ones", 288), ("ident", 128)],
    # fat matmul weights (float32r)
    "R": [("wk1", 512), ("wk2", 512), ("wcat", 1024),
          ("s0", 8), ("s1", 8), ("e0", 128), ("e1", 128)],
    # the rest (fp32, needed later)
    "B": [("wq1", 512), ("bq1", 2), ("wq2", 512), ("bq2", 2),
          ("bk1", 2), ("bk2", 2), ("wout", 512), ("bout", 2)],
}


def wblob_layout(which):
    a = Alloc()
    for nm, wd in WBLOBS[which]:
        a.add(nm, wd)
    return a


NAME2BLOB = {nm: which for which, items in WBLOBS.items() for nm, _ in items}


def xblob_layout():
    a = Alloc()
    for nm, wd in [("deT", 72), ("qsT", 72), ("rpx", 36), ("rpy", 36),
                   ("rpexp", 288)]:
        a.add(nm, wd)
    return a


def pack_wblobs(weights):
    """weights: dict of numpy arrays (original reference layouts)."""
    lays = {w: wblob_layout(w) for w in WBLOBS}
    wbs = {w: np.zeros((128, lays[w].pos), np.float32) for w in WBLOBS}

    def put(name, arr, rows=128, coloff=0):
        lay = lays[NAME2BLOB[name]]; wb = wbs[NAME2BLOB[name]]
        s, _ = lay[name]
        wb[:rows, s + coloff: s + coloff + arr.shape[1]] = arr

    def put_mm(name, w256):  # (256, Mout) -> blocks (kc, mc) of (128, 128)
        lay = lays[NAME2BLOB[name]]; wb = wbs[NAME2BLOB[name]]
        s, _ = lay[name]
        mcs = w256.shape[1] // 128
        for kc in range(2):
            for mc in range(mcs):
                blk = w256[kc * 128:(kc + 1) * 128, mc * 128:(mc + 1) * 128]
                off = (kc * mcs + mc) * 128
                wb[:, s + off: s + off + 128] = blk

    put_mm("wconq", weights["W_con_q"])
    put("bconq", weights["b_con_q"].reshape(2, 128).T)
    # block-diag Wo1 consts: j = cc*2+h2 covers groups (2j, 2j+1)
    s, _ = lays["A"]["bdh"]
    wo1 = weights["Wo1"]  # (32, 64)
    for j in range(4):
        blk = np.zeros((128, 128), np.float32)
        blk[0:32, 0:64] = wo1 if j % 2 == 0 else 0
        if j % 2 == 0:
            blk[0:32, 0:64] = wo1
            blk[32:64, 64:128] = wo1
        else:
            blk[64:96, 0:64] = wo1
            blk[96:128, 64:128] = wo1
        wbs["A"][:, s + j * 128: s + (j + 1) * 128] = blk
    put("bo1rep", np.tile(weights["bo1"], 2)[:, None])
    wo2 = weights["Wo2"]  # (64, 2)
    top = np.zeros((128, 2), np.float32); top[0:64] = wo2
    bot = np.zeros((128, 2), np.float32); bot[64:128] = wo2
    put("wo2top", top); put("wo2bot", bot)
    put("bo2", weights["bo2"][:, None], rows=2)
    put("sc2pm", np.tile(np.array([[PIX_SCALE, -PIX_SCALE]], np.float32),
                         (128, 1)))
    i64 = np.arange(128) // 2
    freq = (TWO_PI / (10000.0 ** (i64 / 64.0))).astype(np.float32)
    put("freq", freq[None, :], rows=1)
    fx2 = np.zeros((2, 128), np.float32); fx2[0] = freq
    fy2 = np.zeros((2, 128), np.float32); fy2[1] = freq
    put("freqx2", fx2, rows=2)
    put("freqy2", fy2, rows=2)
    shift = np.where(np.arange(128) % 2 == 1, np.pi / 2, 0.0).astype(np.float32)
    put("shift", shift[None, :], rows=1)
    put("ones", np.ones((1, 288), np.float32), rows=1)
    put("ident", np.eye(128, dtype=np.float32))
    put_mm("wq1", weights["Wq1"]); put("bq1", weights["bq1"].reshape(2, 128).T)
    put_mm("wq2", weights["Wq2"]); put("bq2", weights["bq2"].reshape(2, 128).T)
    put_mm("wk1", weights["Wk1"]); put("bk1", weights["bk1"].reshape(2, 128).T)
    put_mm("wk2", weights["Wk2"]); put("bk2", weights["bk2"].reshape(2, 128).T)
    wcat = np.concatenate([weights["W_con_k"], weights["W_v"]], axis=1)  # (256,512)
    put_mm("wcat", wcat)
    put_mm("wout", weights["W_out"])
    put("bout", weights["b_out"].reshape(2, 128).T)
    d = np.arange(128)
    s0 = np.zeros((128, 8), np.float32)
    s0[d, d // 32] = SCALE
    s1 = np.zeros((128, 8), np.float32)
    s1[d, 4 + d // 32] = SCALE
    put("s0", s0); put("s1", s1)
    e0 = np.zeros((8, 128), np.float32)
    e0[d // 32, d] = 1.0
    e1 = np.zeros((8, 128), np.float32)
    e1[4 + d // 32, d] = 1.0
    put("e0", e0, rows=8); put("e1", e1, rows=8)
    return wbs


def pack_xblob(dec_embed, query_scale, ref_points, b, k0):
    """Per-core input blob: 36 queries = modes k0..k0+2, all T."""
    lay = xblob_layout()
    xb = np.zeros((128, lay.pos), np.float32)
    de = dec_embed[k0:k0 + 3, b].reshape(NQ, DIM)       # (36, 256)
    qs = query_scale[k0:k0 + 3, b].reshape(NQ, DIM)
    rp = ref_points[k0:k0 + 3, b].reshape(NQ, 2)

    s, _ = lay["deT"]
    xb[:, s: s + 36] = de.T[:128]
    xb[:, s + 36: s + 72] = de.T[128:]
    s, _ = lay["qsT"]
    xb[:, s: s + 36] = qs.T[:128]
    xb[:, s + 36: s + 72] = qs.T[128:]
    s, _ = lay["rpx"]
    xb[0, s: s + 36] = rp[:, 0]
    s, _ = lay["rpy"]
    xb[0, s: s + 36] = rp[:, 1]
    s, _ = lay["rpexp"]
    xb[0:2, s: s + 288] = np.tile(rp.T, (1, 8))         # g-major: col = g*36+q
    return xb


# --------------------------------------------------------------- tile patches

def _split_drain_and_barrier(self, tick_clock, wait_clock):
    nc = self.nc
    drain_inst = nc.sync.drain()
    wait_clock.add_sem_waits(
        drain_inst.ins, tile_mod.ScopedClock({None: tick_clock.global_clock})
    )
    si = drain_inst.ins.sync_info
    waits = list(si.on_wait)
    if len(waits) > 1:
        si.on_wait = waits[:1]
        for i in range(1, len(waits)):
            extra = nc.sync.drain()
            extra.ins.sync_info = type(si)(on_wait=waits[i: i + 1], on_update=[])
    nc.all_engine_barrier()
    assert self.sems is not None
    popped = nc._tile_sem_poison_stack.pop()
    assert popped is self._sem_poison
    nc.clear_and_free_semaphores(list(self.sems.allocated().values()))
    nc.all_engine_barrier()


def split_multiwaits(nc):
    """walrus codegen supports a single sync-wait per instruction; split."""
    f = nc.m.functions[0]
    for blk in f.blocks:
        todo = [i for i in blk.instructions
                if i.sync_info is not None and len(i.sync_info.on_wait) > 1]
        for inst in todo:
            si = inst.sync_info
            waits = list(si.on_wait)
            nops = []
            for w in waits[:-1]:
                bi = nc.engines[inst.engine].nop(nofuse=True)
                ni = bi.ins
                for b2 in f.blocks:
                    if b2.instructions and b2.instructions[-1] is ni:
                        b2.instructions.pop()
                        break
                ni.sync_info = type(si)(on_wait=[w], on_update=[])
                nops.append(ni)
            si.on_wait = [waits[-1]]
            pos = blk.instructions.index(inst)
            blk.instructions[pos:pos] = nops


_PATCHED = False


def patch_tile():
    global _PATCHED
    if not _PATCHED:
        tile_mod.TileContext._drain_and_barrier = _split_drain_and_barrier
        _PATCHED = True


# ---------------------------------------------------------------- the kernel

def view3(ap, dims):
    """Build a 3D AP view on top of a 2D tile AP: dims = [[step,count],...]
    applied after the partition dim (ap.ap[0] kept)."""
    return AP(ap.tensor, ap.offset, [ap.ap[0]] + dims)


def build_nc(sim_mode=False, debug=False):
    patch_tile()
    nc = bass.Bass("TRN2")
    wlays = {w: wblob_layout(w) for w in WBLOBS}
    xlay = xblob_layout()

    bev = nc.dram_tensor("bev", [H * W, 256], F32, kind="ExternalInput")
    wblA = nc.dram_tensor("wblA", [128, wlays["A"].pos], F32, kind="ExternalInput")
    wblR = nc.dram_tensor("wblR", [128, wlays["R"].pos], F32R, kind="ExternalInput")
    wblB = nc.dram_tensor("wblB", [128, wlays["B"].pos], F32, kind="ExternalInput")
    xbl = nc.dram_tensor("xbl", [128, xlay.pos], F32, kind="ExternalInput")
    out = nc.dram_tensor("out", [256, NQ], F32, kind="ExternalOutput")

    dbg = {}
    if debug:
        for nm, shp, dt in [
            ("d_vg", [2, 288], F32), ("d_pix", [128, 2], F32),
            ("d_idx", [128, 1], I32), ("d_sam0", [128, 256], F32),
            ("d_sim", [8, 288], F32), ("d_at", [8, 288], F32R),
            ("d_kse0", [128, 288], F32R), ("d_posk0", [128, 288], F32),
            ("d_conv0", [128, 288], F32), ("d_qse0", [128, 36], F32),
            ("d_cq0", [128, 36], F32), ("d_h", [128, 144], F32),
            ("d_av0", [128, 36], F32), ("d_w40", [128, 4], F32),
        ]:
            dbg[nm] = nc.dram_tensor(nm, shp, dt, kind="ExternalOutput")

    ERF = AF.Sigmoid if sim_mode else AF.Erf

    with tile_mod.TileContext(nc) as tc:
        with (
            tc.tile_pool(name="sbuf", bufs=1) as pool,
            tc.tile_pool(name="psum", bufs=1, space="PSUM") as psum,
        ):
            # warm the {erf,tanh} activation table during the weight DMA
            wt = pool.tile([1, 1], F32)
            nc.vector.memset(wt[:], 0.0)
            warm = pool.tile([1, 1], F32)
            nc.scalar.activation(out=warm[:], in_=wt[:],
                                 func=AF.Sigmoid if sim_mode else AF.Erf,
                                 bias=0.0)

            xb = pool.tile([128, xlay.pos], F32)
            nc.sync.dma_start(out=xb[:], in_=xbl[:])
            wbA = pool.tile([128, wlays["A"].pos], F32)
            nc.sync.dma_start(out=wbA[:], in_=wblA[:])
            wbR = pool.tile([128, wlays["R"].pos], F32R)
            nc.sync.dma_start(out=wbR[:], in_=wblR[:])
            wbB = pool.tile([128, wlays["B"].pos], F32)
            nc.sync.dma_start(out=wbB[:], in_=wblB[:])
            wbtiles = {"A": wbA, "R": wbR, "B": wbB}

            def wsl(name, rows=128, off=0, width=None):
                which = NAME2BLOB[name]
                s, wd = wlays[which][name]
                if width is None:
                    width = wd - off
                return wbtiles[which][0:rows, s + off: s + off + width]

            def xsl(name, rows=128, off=0, width=None):
                s, wd = xlay[name]
                if width is None:
                    width = wd - off
                return xb[0:rows, s + off: s + off + width]

            deT = [xsl("deT", off=mc * 36, width=36) for mc in range(2)]
            qsT = [xsl("qsT", off=mc * 36, width=36) for mc in range(2)]

            # ---- 1. con_q = de @ W_con_q + b  (feature-major, 2 chunks)
            cqS = []
            for mc in range(2):
                p = psum.tile([128, 288], F32, space="PSUM", tag="psA", bufs=3, name="cqP")
                for kc in range(2):
                    nc.tensor.matmul(
                        out=p[:, :36], lhsT=wsl("wconq", off=(kc * 2 + mc) * 128, width=128),
                        rhs=deT[kc], start=(kc == 0), stop=(kc == 1))
                t = pool.tile([128, 36], F32, tag=f"cqS{mc}")
                nc.scalar.activation(out=t[:], in_=p[:, :36], func=AF.Identity,
                                     bias=wsl("bconq", off=mc, width=1))
                cqS.append(t)
            if debug:
                nc.sync.dma_start(out=dbg["d_cq0"][:], in_=cqS[0][:])

            # ---- 2. h = gelu(grouped con_q @ Wo1 + bo1): 4 block-diag mms
            hP = psum.tile([128, 288], F32, space="PSUM", tag="psA", bufs=3, name="hP")
            for j in range(4):
                cc = j // 2
                nc.tensor.matmul(
                    out=hP[:, j * 36:(j + 1) * 36],
                    lhsT=wsl("bdh", off=j * 128, width=128),
                    rhs=cqS[cc][:], start=True, stop=True)
            hx = pool.tile([128, 144], F32)
            nc.scalar.activation(out=hx[:], in_=hP[:, :144], func=AF.Identity,
                                 bias=wsl("bo1rep"))
            he = pool.tile([128, 144], F32)
            nc.scalar.activation(out=he[:], in_=hx[:], func=ERF,
                                 scale=float(1 / np.sqrt(2)), bias=0.0)
            nc.vector.tensor_scalar(out=he[:], in0=he[:], scalar1=0.5,
                                    scalar2=0.5, op0=OP.mult, op1=OP.add)
            hS = pool.tile([128, 144], F32)
            nc.vector.tensor_tensor(out=hS[:], in0=hx[:], in1=he[:], op=OP.mult)
            if debug:
                nc.sync.dma_start(out=dbg["d_h"][:], in_=hS[:])

            # ---- 3. offsets + vgrid (meters), g-major (2, 288)
            offP = psum.tile([2, 288], F32, space="PSUM", tag="psA", bufs=3, name="offP")
            for g in range(8):
                j = g // 2
                lhs = wsl("wo2top", width=2) if g % 2 == 0 else wsl("wo2bot", width=2)
                nc.tensor.matmul(out=offP[:, g * 36:(g + 1) * 36], lhsT=lhs,
                                 rhs=hS[:, j * 36:(j + 1) * 36],
                                 start=True, stop=True)
            tof = pool.tile([2, 288], F32)
            nc.scalar.activation(out=tof[:], in_=offP[:], func=AF.Tanh,
                                 bias=wsl("bo2", rows=2, width=1))
            vgT = pool.tile([2, 288], F32)
            nc.vector.tensor_scalar(out=vgT[:], in0=tof[:], scalar1=OFFSET_SCALE,
                                    scalar2=None, op0=OP.mult)
            nc.vector.tensor_tensor(out=vgT[:], in0=vgT[:],
                                    in1=xsl("rpexp", rows=2), op=OP.add)
            if debug:
                nc.sync.dma_start(out=dbg["d_vg"][:], in_=vgT[:])

            # ---- 4+5. transpose vgrid to point-major, then per-point geometry
            # (all per-point scalars live in columns of the same partition)
            idxI, w4, pixdbg = [], [], None
            for c, (c0, cn) in enumerate(CHUNKS):
                tp = psum.tile([128, 2], F32, space="PSUM", tag="psA", bufs=3, name="tpP")
                nc.tensor.transpose(out=tp[:cn, :], in_=vgT[:, c0:c0 + cn],
                                    identity=wsl("ident", rows=2, width=2))
                # pix = vg * [s, -s] + 99.5   (cols [gx, gy])
                pix = pool.tile([128, 2], F32, tag=f"pix{c}")
                nc.vector.tensor_tensor(out=pix[:cn, :], in0=tp[:cn, :],
                                        in1=wsl("sc2pm", rows=cn, width=2),
                                        op=OP.mult)
                nc.vector.tensor_scalar(out=pix[:cn, :], in0=pix[:cn, :],
                                        scalar1=PIX_BIAS, scalar2=None,
                                        op0=OP.add)
                # f0 = rint(pix - 0.5) = floor(pix) via the 1.5*2^23 trick
                f0 = pool.tile([128, 2], F32, tag=f"f0{c}")
                nc.vector.tensor_scalar(out=f0[:cn, :], in0=pix[:cn, :],
                                        scalar1=-0.5, scalar2=float(RC),
                                        op0=OP.add, op1=OP.add)
                nc.vector.tensor_scalar(out=f0[:cn, :], in0=f0[:cn, :],
                                        scalar1=float(-RC), scalar2=None,
                                        op0=OP.add)
                fr = pool.tile([128, 2], F32, tag=f"fr{c}")
                nc.vector.tensor_tensor(out=fr[:cn, :], in0=pix[:cn, :],
                                        in1=f0[:cn, :], op=OP.subtract)
                # idx = y0*200 + x0 (float-exact, then cast)
                idf = pool.tile([128, 1], F32, tag=f"idf{c}")
                nc.vector.tensor_scalar(out=idf[:cn, :], in0=f0[:cn, 1:2],
                                        scalar1=float(W), scalar2=None,
                                        op0=OP.mult)
                nc.vector.tensor_tensor(out=idf[:cn, :], in0=idf[:cn, :],
                                        in1=f0[:cn, 0:1], op=OP.add)
                ii = pool.tile([128, 1], I32, tag=f"idxI{c}")
                nc.vector.tensor_copy(out=ii[:cn, :], in_=idf[:cn, :])
                idxI.append(ii)
                # bilinear weights (Pc, 4) = [w00, w10, w01, w11]
                wxp = pool.tile([128, 2], F32, tag=f"wxp{c}")
                nc.vector.tensor_scalar(out=wxp[:cn, 0:1], in0=fr[:cn, 0:1],
                                        scalar1=-1.0, scalar2=1.0,
                                        op0=OP.mult, op1=OP.add)
                nc.scalar.copy(out=wxp[:cn, 1:2], in_=fr[:cn, 0:1])
                wyp = pool.tile([128, 2], F32, tag=f"wyp{c}")
                nc.vector.tensor_scalar(out=wyp[:cn, 0:1], in0=fr[:cn, 1:2],
                                        scalar1=-1.0, scalar2=1.0,
                                        op0=OP.mult, op1=OP.add)
                nc.scalar.copy(out=wyp[:cn, 1:2], in_=fr[:cn, 1:2])
                w4c = pool.tile([128, 4], F32, tag=f"w4{c}")
                wxa = wxp[:cn, :]
                wya = wyp[:cn, :]
                nc.vector.tensor_tensor(
                    out=view3(w4c[:cn, :], [[2, 2], [1, 2]]),
                    in0=AP(wxa.tensor, wxa.offset, [wxa.ap[0], [0, 2], [1, 2]]),
                    in1=AP(wya.tensor, wya.offset, [wya.ap[0], [1, 2], [0, 2]]),
                    op=OP.mult)
                w4.append(w4c)
                if debug and c == 0:
                    pixdbg = pix
            if debug:
                nc.sync.dma_start(out=dbg["d_pix"][:], in_=pixdbg[:])
                nc.sync.dma_start(out=dbg["d_idx"][:], in_=idxI[0][:])
                nc.sync.dma_start(out=dbg["d_w40"][:], in_=w4[0][:])

            # ---- 6. gathers: 2KB rows y0 / y0+1 per point
            gA, gB = [], []
            for c, (c0, cn) in enumerate(CHUNKS):
                ga = pool.tile([128, 512], F32, tag=f"gA{c}")
                nc.gpsimd.indirect_dma_start(
                    out=ga[:cn, :], out_offset=None, in_=bev[:],
                    in_offset=IndirectOffsetOnAxis(ap=idxI[c][:cn, :], axis=0))
                gb = pool.tile([128, 512], F32, tag=f"gB{c}")
                nc.gpsimd.indirect_dma_start(
                    out=gb[:cn, :], out_offset=None, in_=bev[:],
                    in_offset=IndirectOffsetOnAxis(ap=idxI[c][:cn, :], axis=0),
                    element_offset=W * 256)
                gA.append(ga); gB.append(gb)

            # ---- 7. bilinear combine -> sampled (point-major)
            sam = []
            for c, (c0, cn) in enumerate(CHUNKS):
                t1 = pool.tile([128, 256], F32, tag=f"bt1{c}")
                t2 = pool.tile([128, 256], F32, tag=f"bt2{c}")
                sm = pool.tile([128, 256], F32, tag=f"sam{c}")
                nc.scalar.activation(out=t1[:cn, :], in_=gA[c][:cn, 0:256],
                                     func=AF.Copy, scale=w4[c][:cn, 0:1])
                nc.vector.tensor_scalar(out=t2[:cn, :], in0=gA[c][:cn, 256:512],
                                        scalar1=w4[c][:cn, 1:2], scalar2=None,
                                        op0=OP.mult)
                nc.vector.tensor_tensor(out=t1[:cn, :], in0=t1[:cn, :],
                                        in1=t2[:cn, :], op=OP.add)
                nc.scalar.activation(out=t2[:cn, :], in_=gB[c][:cn, 0:256],
                                     func=AF.Copy, scale=w4[c][:cn, 2:3])
                nc.vector.tensor_tensor(out=t1[:cn, :], in0=t1[:cn, :],
                                        in1=t2[:cn, :], op=OP.add)
                nc.vector.tensor_scalar(out=t2[:cn, :], in0=gB[c][:cn, 256:512],
                                        scalar1=w4[c][:cn, 3:4], scalar2=None,
                                        op0=OP.mult)
                nc.vector.tensor_tensor(out=sm[:cn, :], in0=t1[:cn, :],
                                        in1=t2[:cn, :], op=OP.add)
                sam.append(sm)
            if debug:
                nc.sync.dma_start(out=dbg["d_sam0"][:], in_=sam[0][:])

            # ---- 8. transpose sampled to feature-major (256, 288) = 2 tiles
            samT = [pool.tile([128, 288], F32R, tag=f"samT{fc}", name=f"samT{fc}")
                    for fc in range(2)]
            for c, (c0, cn) in enumerate(CHUNKS):
                for fc in range(2):
                    tp = psum.tile([128, 128], F32, space="PSUM", tag="psA", bufs=3, name="samtp")
                    nc.tensor.transpose(out=tp[:, :cn],
                                        in_=sam[c][:cn, fc * 128:(fc + 1) * 128],
                                        identity=wsl("ident", rows=cn, width=cn))
                    nc.scalar.copy(out=samT[fc][:, c0:c0 + cn], in_=tp[:, :cn])

            # ---- 9. conv: con_k / v = sampled @ [W_con_k | W_v]
            convP = []
            for mc in range(4):
                p = psum.tile([128, 288], F32, space="PSUM", tag="convP", bufs=4, name="convP")
                for kc in range(2):
                    nc.tensor.matmul(
                        out=p[:], lhsT=wsl("wcat", off=(kc * 4 + mc) * 128, width=128),
                        rhs=samT[kc][:], start=(kc == 0), stop=(kc == 1))
                convP.append(p)
            vS = []
            for fc in range(2):
                t = pool.tile([128, 288], F32, tag=f"vS{fc}")
                nc.scalar.copy(out=t[:], in_=convP[2 + fc][:])
                vS.append(t)
            if debug:
                t = pool.tile([128, 288], F32)
                nc.scalar.copy(out=t[:], in_=convP[0][:])
                nc.sync.dma_start(out=dbg["d_conv0"][:], in_=t[:])

            # ---- helpers: sine embedding (feature-major halves)
            def sine_embed(lhs_name, lhs_rows, rhs_ap, n, tag, odt=F32):
                """phase = freq (x) meters + shift; one 128-row half."""
                ph = psum.tile([128, 288], F32, space="PSUM", tag="psA", bufs=3, name="phP")
                nc.tensor.matmul(out=ph[:, :n], lhsT=wsl(lhs_name, rows=lhs_rows),
                                 rhs=rhs_ap, start=True, stop=False)
                nc.tensor.matmul(out=ph[:, :n], lhsT=wsl("shift", rows=1),
                                 rhs=wsl("ones", rows=1, width=n),
                                 start=False, stop=True)
                m1t = pool.tile([128, n], F32, tag=f"sm1{tag}")
                nc.vector.tensor_scalar(out=m1t[:], in0=ph[:, :n],
                                        scalar1=float(1.0 / TWO_PI), scalar2=RC,
                                        op0=OP.mult, op1=OP.add)
                k2t = pool.tile([128, n], F32, tag=f"sk2{tag}")
                nc.vector.tensor_scalar(out=k2t[:], in0=m1t[:], scalar1=-RC,
                                        scalar2=-TWO_PI, op0=OP.add, op1=OP.mult)
                yt = pool.tile([128, n], F32, tag=f"sy{tag}")
                nc.vector.tensor_tensor(out=yt[:], in0=ph[:, :n], in1=k2t[:], op=OP.add)
                nc.vector.tensor_scalar(out=yt[:], in0=yt[:],
                                        scalar1=float(np.pi),
                                        scalar2=float(-np.pi),
                                        op0=OP.min, op1=OP.max)
                st = pool.tile([128, n], odt, tag=f"se{tag}")
                nc.scalar.activation(out=st[:], in_=yt[:], func=AF.Sin)
                return st

            def mlp2(inp2, n, wn1, bn1, wn2, bn2, tag, middt=F32):
                """two-layer MLP relu(x@W1+b1)@W2+b2, feature-major chunks."""
                mid = []
                for mc in range(2):
                    p = psum.tile([128, 288], F32, space="PSUM", tag="psA", bufs=3, name="m1P")
                    for kc in range(2):
                        nc.tensor.matmul(
                            out=p[:, :n], lhsT=wsl(wn1, off=(kc * 2 + mc) * 128, width=128),
                            rhs=inp2[kc][:], start=(kc == 0), stop=(kc == 1))
                    t = pool.tile([128, n], middt, tag=f"m1S{tag}{mc}")
                    nc.scalar.activation(out=t[:], in_=p[:, :n], func=AF.Relu,
                                         bias=wsl(bn1, off=mc, width=1))
                    mid.append(t)
                outs = []
                for mc in range(2):
                    p = psum.tile([128, 288], F32, space="PSUM", tag="psA", bufs=3, name="m2P")
                    for kc in range(2):
                        nc.tensor.matmul(
                            out=p[:, :n], lhsT=wsl(wn2, off=(kc * 2 + mc) * 128, width=128),
                            rhs=mid[kc][:], start=(kc == 0), stop=(kc == 1))
                    t = pool.tile([128, n], F32, tag=f"m2S{tag}{mc}")
                    nc.scalar.activation(out=t[:], in_=p[:, :n], func=AF.Identity,
                                         bias=wsl(bn2, off=mc, width=1))
                    outs.append(t)
                return outs

            # ---- 10. qse + pos_q (36 cols)  [emb(y) first, then emb(x)]
            qse = [sine_embed("freq", 1, xsl("rpy", rows=1), 36, "qy"),
                   sine_embed("freq", 1, xsl("rpx", rows=1), 36, "qx")]
            if debug:
                nc.sync.dma_start(out=dbg["d_qse0"][:], in_=qse[0][:])
            pqS = mlp2(qse, 36, "wq1", "bq1", "wq2", "bq2", "q")
            for mc in range(2):
                nc.vector.tensor_tensor(out=pqS[mc][:], in0=pqS[mc][:],
                                        in1=qsT[mc], op=OP.mult)

            # ---- 11. kse + pos_k (288 cols)
            kse = [sine_embed("freqy2", 2, vgT[:], 288, "ky", odt=F32R),
                   sine_embed("freqx2", 2, vgT[:], 288, "kx", odt=F32R)]
            if debug:
                nc.sync.dma_start(out=dbg["d_kse0"][:], in_=kse[0][:])
            pkS = mlp2(kse, 288, "wk1", "bk1", "wk2", "bk2", "k", middt=F32R)
            if debug:
                nc.sync.dma_start(out=dbg["d_posk0"][:], in_=pkS[0][:])

            # ---- 12. sim = scaled per-head dots via selection matmuls
            simP = psum.tile([8, 288], F32, space="PSUM", tag="simP", bufs=1, name="simP")
            pairs = [(convP[0], cqS[0], "s0"), (convP[1], cqS[1], "s1"),
                     (pkS[0], pqS[0], "s0"), (pkS[1], pqS[1], "s1")]
            for i, (kpart, qpart, sname) in enumerate(pairs):
                tmp = pool.tile([128, 288], F32R, tag=f"tmp{i}")
                qap = qpart[:]
                nc.vector.tensor_tensor(
                    out=view3(tmp[:], [[36, 8], [1, 36]]),
                    in0=view3(kpart[:], [[36, 8], [1, 36]]),
                    in1=AP(qap.tensor, qap.offset, [qap.ap[0], [0, 8], [1, 36]]),
                    op=OP.mult)
                nc.tensor.matmul(out=simP[:], lhsT=wsl(sname, width=8),
                                 rhs=tmp[:], start=(i == 0), stop=(i == 3))
            if debug:
                t = pool.tile([8, 288], F32)
                nc.vector.tensor_copy(out=t[:], in_=simP[:])
                nc.sync.dma_start(out=dbg["d_sim"][:], in_=t[:])

            # ---- 13. softmax over g (stride-36 views; cols are g-major)
            mx = pool.tile([8, 36], F32)
            nc.vector.reduce_max(out=mx[:], in_=view3(simP[:], [[1, 36], [36, 8]]),
                                 axis=mybir.AxisListType.X)
            ex = pool.tile([8, 288], F32)
            mxa = mx[:]
            nc.vector.tensor_tensor(
                out=view3(ex[:], [[1, 36], [36, 8]]),
                in0=view3(simP[:], [[1, 36], [36, 8]]),
                in1=AP(mxa.tensor, mxa.offset, [mxa.ap[0], [1, 36], [0, 8]]),
                op=OP.subtract)
            nc.scalar.activation(out=ex[:], in_=ex[:], func=AF.Exp)
            sm = pool.tile([8, 36], F32)
            nc.vector.reduce_sum(out=sm[:], in_=view3(ex[:], [[1, 36], [36, 8]]),
                                 axis=mybir.AxisListType.X)
            rc = pool.tile([8, 36], F32)
            nc.vector.reciprocal(out=rc[:], in_=sm[:])
            at = pool.tile([8, 288], F32R)
            rca = rc[:]
            nc.vector.tensor_tensor(
                out=view3(at[:], [[1, 36], [36, 8]]),
                in0=view3(ex[:], [[1, 36], [36, 8]]),
                in1=AP(rca.tensor, rca.offset, [rca.ap[0], [1, 36], [0, 8]]),
                op=OP.mult)
            if debug:
                nc.sync.dma_start(out=dbg["d_at"][:], in_=at[:])

            # ---- 14. attn-weighted values -> (256, 36) feature-major
            avT = []
            for fc in range(2):
                ae = psum.tile([128, 288], F32, space="PSUM", tag="psA", bufs=3, name="aeP")
                nc.tensor.matmul(out=ae[:], lhsT=wsl(f"e{fc}", rows=8, width=128),
                                 rhs=at[:], start=True, stop=True)
                pr = pool.tile([128, 288], F32, tag=f"pr{fc}")
                nc.vector.tensor_tensor(out=pr[:], in0=vS[fc][:], in1=ae[:],
                                        op=OP.mult)
                av = pool.tile([128, 36], F32, tag=f"avT{fc}")
                nc.vector.reduce_sum(out=av[:], in_=view3(pr[:], [[1, 36], [36, 8]]),
                                     axis=mybir.AxisListType.X)
                avT.append(av)
            if debug:
                nc.sync.dma_start(out=dbg["d_av0"][:], in_=avT[0][:])

            # ---- 15. out = attn_out @ W_out + b_out + identity
            for mc in range(2):
                p = psum.tile([128, 288], F32, space="PSUM", tag="psA", bufs=3, name="oP")
                for kc in range(2):
                    nc.tensor.matmul(
                        out=p[:, :36], lhsT=wsl("wout", off=(kc * 2 + mc) * 128, width=128),
                        rhs=avT[kc][:], start=(kc == 0), stop=(kc == 1))
                t = pool.tile([128, 36], F32, tag=f"oS{mc}")
                nc.scalar.activation(out=t[:], in_=p[:, :36], func=AF.Identity,
                                     bias=wsl("bout", off=mc, width=1))
                nc.vector.tensor_tensor(out=t[:], in0=t[:], in1=deT[mc], op=OP.add)
                nc.sync.dma_start(out=out[mc * 128:(mc + 1) * 128, :], in_=t[:])

    return nc


# ------------------------------------------------------------------- driver

def make_in_maps(dec_embed, bev_feat, query_scale, ref_points, weights):
    wbs = pack_wblobs(weights)
    in_maps = []
    for c in range(8):
        b, kh = c // 2, c % 2
        bev_hwc = np.ascontiguousarray(
            bev_feat[b].transpose(1, 2, 0).reshape(H * W, 256))
        xb = pack_xblob(dec_embed, query_scale, ref_points, b, 3 * kh)
        in_maps.append({"bev": bev_hwc, "wblA": wbs["A"], "wblR": wbs["R"],
                        "wblB": wbs["B"], "xbl": xb})
    return in_maps


def assemble_output(results, dec_dtype=np.float32):
    out = np.zeros((K, B, T, DIM), np.float32)
    for c in range(8):
        b, kh = c // 2, c % 2
        oc = results[c]["out"]                     # (256, 36)
        out[3 * kh:3 * kh + 3, b] = oc.T.reshape(3, T, DIM)
    return out


_WNAMES = ["W_con_q", "b_con_q", "W_con_k", "W_v", "Wq1", "bq1", "Wq2", "bq2",
           "Wk1", "bk1", "Wk2", "bk2", "Wo1", "bo1", "Wo2", "bo2",
           "W_out", "b_out"]


def kernel(**inputs):
    from concourse.bass_utils import run_bass_kernel_spmd
    dec_embed = np.asarray(inputs["dec_embed"], np.float32)
    bev_feat = np.asarray(inputs["bev_feat"], np.float32)
    query_scale = np.asarray(inputs["query_scale"], np.float32)
    ref_points = np.asarray(inputs["ref_points"], np.float32)
    weights = {n: np.asarray(inputs[n], np.float32) for n in _WNAMES}

    nc = build_nc(sim_mode=False, debug=False)
    split_multiwaits(nc)
    in_maps = make_in_maps(dec_embed, bev_feat, query_scale, ref_points, weights)
    res = run_bass_kernel_spmd(nc, in_maps, list(range(8)))
    return assemble_output(res.results)
